# revision 63
# baseline (speedup 1.0000x reference)
"""Trainium2 Bass kernel for nn_AttentionModule (dense_transformer), fp8 DR.

Reference computation (per batch sample b):
    theta = sigmoid(x @ Wt + bt)            # [N, F]
    phi   = x @ Wp + bp                     # [N, F]
    att   = theta @ phi.T                   # [N(n), N(m)]
    att   = softmax(att, axis over n)       # softmax over QUERY axis
    out   = att(n,m) @ x(m,d) + x           # [N, D]

Strategy: pure data parallelism (B=8 samples, one per core, no
collectives) + fp8-e4m3 DoubleRow matmuls (0.5 cycles/row, 2x128
contraction per instruction = 4x bf16 MAC rate in the cost model).

Precision scheme (validated in numerics2.py, rel-l2 vs f64 ref):
 - every bf16 matmul X@W is replaced by 2-3 fp8 terms
       X8@W8 + X8@Wr + Xr@W8      (r = unscaled e4m3 residual)
   accumulated in the same fp32 PSUM group; residual pairs restore
   ~bf16-equivalent precision at 0.5-0.75x of bf16 matmul time.
 - weights are pre-scaled by 32 on host (W' = 32W) so their e4m3
   residuals land in normal range; the 1/32 is folded into the
   activation scale.
 - theta is carried as T = tanh(z/2) = 2*sigmoid(z)-1 in (-1,1):
   the scores logit becomes 0.5*(T.phi) + 0.5*sum_f(phi[m]); the
   second term is constant per m and cancels in the softmax over n
   (exp bias -20; measured exponent range on real inputs [-8, 11.1]).
 - att is quantized to e4m3 AFTER normalization; weighted uses
   2 terms (att8@x8 + att8@xr8, rel~1.38e-2) or 3 terms (+Ar@x8,
   rel~5.5e-3) per WEIGHTED_TERMS.

Scheduling (engine budgets from the TimelineSim cost model):
 - PE (full clock): proj 41us + scores 41us + weighted 55us (2t).
 - PSUM tiles are 2 banks wide; two 512-wide accumulation groups
   share a tile so phase-2 exp runs once per ns-PAIR ([128,1024]
   reads). This halves the per-exp fixed costs (172-cycle PSUM
   access + 187ns accumulator read) that otherwise pace phase 2
   above PE rate (ACT was 799ns/group vs PE 642ns).
 - exp uses accum_out for free row-sums; normalization (att8) runs
   on DVE; residual prep is split ACT/DVE so both stay under PE.
 - tiles are split to consumer granularity (deps are tile-granular):
   W per fc (startup pipelining), T/P per (fc-pair, ns), att8/Ar per
   m-chunk-pair, E per chunk (rotating pool).
 - SBUF: the phase-1 operand pool is closed after phase 1 and its
   space reused for the phase-3 x tiles (LIFO pool stack).
"""

import numpy as np
import ml_dtypes

import concourse.bass as bass
import concourse.bacc as bacc
import concourse.mybir as mybir
from concourse.tile import TileContext
from concourse.bass_utils import run_bass_kernel_spmd

P = 128
B, N, D, F = 8, 2048, 1024, 512
NCH = N // P    # 16 m/n chunks
DCH = D // P    # 8 d chunks
FCH = F // P    # 4 f chunks
NF = 512        # accumulation-group width (half a 2-bank psum tile)
NSL = N // NF   # 4 column slices
DSL = D // NF   # 2 output d slices
WSCALE = 32.0   # host weight pre-scale (residuals out of denormals)

WEIGHTED_TERMS = 2   # 2: att8@(x8+xr8) ~1.4e-2 | 3: +Ar@x8 ~5.5e-3

BF16 = mybir.dt.bfloat16
F32 = mybir.dt.float32
E4 = mybir.dt.float8e4
AX = mybir.AxisListType.X
AF = mybir.ActivationFunctionType
DR = mybir.MatmulPerfMode.DoubleRow
ALU = mybir.AluOpType
E4NP = ml_dtypes.float8_e4m3


def build_bass():
    nc = bacc.Bacc()

    # startup image: what the first (fc0, ns0) groups' main+wres terms need,
    # in ONE DMA: [wt80|wtr80|wp80|wpr80|xt80] packed per dc row. xtr80
    # (the xres operand, LAST term of each group) rides separately: the
    # scheduler's wait-queue bypass covers ~0.9us of main/wres work while
    # it lands, so phase 1 starts ~1.2us earlier.
    SUW = 4 * P + NF
    su_d = nc.declare_dram_parameter("su", [P, DCH, SUW], E4, isOutput=False)
    xtr0_d = nc.declare_dram_parameter("xtr80_", [P, DCH, NF], E4,
                                       isOutput=False)
    wt8r_d = nc.declare_dram_parameter("wt8r", [P, FCH - 1, DCH, P], E4,
                                       isOutput=False)
    wtrr_d = nc.declare_dram_parameter("wtr8r", [P, FCH - 1, DCH, P], E4,
                                       isOutput=False)
    wp8r_d = nc.declare_dram_parameter("wp8r", [P, FCH - 1, DCH, P], E4,
                                       isOutput=False)
    wprr_d = nc.declare_dram_parameter("wpr8r", [P, FCH - 1, DCH, P], E4,
                                       isOutput=False)
    bias_d = nc.declare_dram_parameter("bias", [P, 2, FCH], F32,
                                       isOutput=False)
    xt8_d = [nc.declare_dram_parameter(f"xt8{ns}", [P, DCH, NF], E4,
                                       isOutput=False) for ns in range(1, NSL)]
    xtr_d = [nc.declare_dram_parameter(f"xtr8{ns}", [P, DCH, NF], E4,
                                       isOutput=False) for ns in range(1, NSL)]
    xn8_d = nc.declare_dram_parameter("xn8", [P, NCH, D], E4, isOutput=False)
    xnr_d = nc.declare_dram_parameter("xnr8", [P, NCH, D], E4, isOutput=False)
    xr_d = nc.declare_dram_parameter("xr", [N, D], BF16, isOutput=False)
    # bf16 output (upcast on host): halves the output DMA traffic and the
    # post-PE drain; adds ~2e-4 rel error (negligible vs the 1.4e-2 total)
    out_d = nc.declare_dram_parameter("out", [N, D], BF16, isOutput=True)

    with TileContext(nc) as tc:
        from contextlib import ExitStack
        es = ExitStack()
        cpool = es.enter_context(tc.tile_pool(name="const", bufs=1))
        stats = es.enter_context(tc.tile_pool(name="stats", bufs=8))
        epool = es.enter_context(tc.tile_pool(name="ep", bufs=8))
        apool = es.enter_context(tc.tile_pool(name="a8", bufs=1))
        appool = es.enter_context(tc.tile_pool(name="apre", bufs=2))
        xrp = es.enter_context(tc.tile_pool(name="xrp", bufs=3))
        outp = es.enter_context(tc.tile_pool(name="outp", bufs=3))
        tpp = es.enter_context(tc.tile_pool(name="tpp", bufs=1))
        tst = es.enter_context(tc.tile_pool(name="tst", bufs=6))
        # 2-bank psum tiles, two 512-wide groups per tile (zero regions are
        # 2KB, so each half is an independent accumulation group)
        psum = es.enter_context(tc.tile_pool(name="psum", bufs=4,
                                             space="PSUM"))
        ph1cm = tc.tile_pool(name="ph1", bufs=1)
        ph1 = ph1cm.__enter__()

        def ptile():
            return psum.tile([P, 2 * NF], F32, name="pst", tag="pst")

        # ---- constants ----
        bias_s = cpool.tile([P, 2, FCH], F32, name="bias", tag="bias")
        bt2_s = bias_s[:, 0]
        bp_s = bias_s[:, 1]
        zx = cpool.tile([P, P], BF16, name="zx", tag="zx")
        nc.vector.memset(zx, 0)

        # PE warm-up: the pstate ramp holds PE below 2.4GHz for ~3.4us of
        # sustained activity; the first real matmul waits on DMA anyway, so
        # burn the idle time on dummy matmuls (costless: PE was idle).
        NWARM = 46  # 128-wide dummies: cover the startup DMA latency; slight
        # overshoot is safer than a gap (a PE gap resets the pstate ramp)
        zp = ptile()
        for i in range(NWARM):
            nc.tensor.matmul(zp[:, 0:P], zx, zx, start=(i == 0),
                             stop=(i == NWARM - 1))
        eb_s = cpool.tile([P, 1], F32, name="ebs", tag="ebs")
        nc.vector.memset(eb_s, -20.0)

        # ---- phase-1 operand tiles + DMAs (first-use order) ----
        su_s = ph1.tile([P, DCH, SUW], E4, name="sus", tag="sus")
        xtr0_s = ph1.tile([P, DCH, NF], E4, name="xtr0s", tag="xtr0s")
        wt8r_s = ph1.tile([P, FCH - 1, DCH, P], E4, name="wt8rs", tag="wt8rs")
        wtrr_s = ph1.tile([P, FCH - 1, DCH, P], E4, name="wtrrs", tag="wtrrs")
        wp8r_s = ph1.tile([P, FCH - 1, DCH, P], E4, name="wp8rs", tag="wp8rs")
        wprr_s = ph1.tile([P, FCH - 1, DCH, P], E4, name="wprrs", tag="wprrs")
        xt8_s = [None] + [ph1.tile([P, DCH, NF], E4, name=f"xt8{ns}",
                                   tag=f"xt8{ns}") for ns in range(1, NSL)]
        xtr_s = [None] + [ph1.tile([P, DCH, NF], E4, name=f"xtr{ns}",
                                   tag=f"xtr{ns}") for ns in range(1, NSL)]

        def wt_ap(fc, s2):
            return su_s[:, s2, 0:P] if fc == 0 else wt8r_s[:, fc - 1, s2]

        def wtr_ap(fc, s2):
            return (su_s[:, s2, P:2 * P] if fc == 0
                    else wtrr_s[:, fc - 1, s2])

        def wp_ap(fc, s2):
            return (su_s[:, s2, 2 * P:3 * P] if fc == 0
                    else wp8r_s[:, fc - 1, s2])

        def wpr_ap(fc, s2):
            return (su_s[:, s2, 3 * P:4 * P] if fc == 0
                    else wprr_s[:, fc - 1, s2])

        def xt_mov(ns, s2):
            return (su_s[:, s2, 4 * P:4 * P + NF] if ns == 0
                    else xt8_s[ns][:, s2])

        def xtr_mov(ns, s2):
            return xtr0_s[:, s2] if ns == 0 else xtr_s[ns][:, s2]

        nc.sync.dma_start(out=su_s, in_=su_d[:])
        nc.sync.dma_start(out=xtr0_s, in_=xtr0_d[:])
        nc.sync.dma_start(out=bias_s, in_=bias_d[:])
        nc.sync.dma_start(out=wt8r_s, in_=wt8r_d[:])
        nc.sync.dma_start(out=wtrr_s, in_=wtrr_d[:])
        nc.sync.dma_start(out=wp8r_s, in_=wp8r_d[:])
        nc.sync.dma_start(out=wprr_s, in_=wprr_d[:])
        for ns in range(1, NSL):
            nc.sync.dma_start(out=xt8_s[ns], in_=xt8_d[ns - 1][:])
            nc.sync.dma_start(out=xtr_s[ns], in_=xtr_d[ns - 1][:])

        # T/P operand tiles: per (fc-pair, ns-512) so scores(mc, ns) only
        # waits on the exact phase-1 blocks it reads.
        FCP = FCH // 2
        t8_s = [[tpp.tile([P, 2, NF], E4, name=f"t8_{j}_{ns}",
                          tag=f"t8_{j}_{ns}") for ns in range(NSL)]
                for j in range(FCP)]
        tr_s = [[tpp.tile([P, 2, NF], E4, name=f"tr_{j}_{ns}",
                          tag=f"tr_{j}_{ns}") for ns in range(NSL)]
                for j in range(FCP)]
        p8_s = [[tpp.tile([P, 2, NF], E4, name=f"p8_{j}_{ns}",
                          tag=f"p8_{j}_{ns}") for ns in range(NSL)]
                for j in range(FCP)]
        pr_s = [[tpp.tile([P, 2, NF], E4, name=f"pr_{j}_{ns}",
                          tag=f"pr_{j}_{ns}") for ns in range(NSL)]
                for j in range(FCP)]

        # -------- Phase 1: projections (3-term fp8 DR) --------
        # psum = x8@W8' + x8@Wr' + xr8@W8'   (W' = 32W; 12 DR per group)
        # T = tanh(psum/64 + bt/2)  -> bf16 staging + e4m3 + residual
        # phi = psum/32 + bp        -> same
        pt = None
        for ns in range(NSL):
            # theta/phi interleaved: ACT sees tanh,tanh,id,... instead of an
            # 8-tanh bunch followed by 4 ids — smoother consumer flow for the
            # psum-tile rotation
            for gi, (proj, fc) in enumerate(
                    [(pr_, fc) for fc in range(FCH) for pr_ in ("t", "p")]):
                pt = ptile()
                ps = pt[:, 0:NF]
                wa, wra = ((wt_ap, wtr_ap) if proj == "t"
                           else (wp_ap, wpr_ap))
                # term-major order: all main-term matmuls first, so the
                # startup groups run while the residual operands' DMAs land
                nmm = 0
                for (wf, xf) in ((wa, xt_mov), (wra, xt_mov),
                                 (wa, xtr_mov)):
                    for dcp in range(DCH // 2):
                        s2 = slice(2 * dcp, 2 * dcp + 2)
                        nc.tensor.matmul(ps, wf(fc, s2), xf(ns, s2),
                                         start=(nmm == 0), stop=(nmm == 11),
                                         perf_mode=DR)
                        nmm += 1
                j, h = fc // 2, fc % 2
                if proj == "t":
                    tprec = tst.tile([P, NF], BF16, name="tpr", tag="tpr")
                    nc.scalar.activation(tprec, ps, AF.Tanh,
                                         bias=bt2_s[:, fc:fc + 1],
                                         scale=1.0 / (2 * WSCALE))
                    # t8 from the bf16 staging on DVE (not a 2nd ACT tanh):
                    # frees the psum bank after ONE ACT pass
                    nc.vector.tensor_copy(t8_s[j][ns][:, h], tprec)
                    nc.vector.tensor_sub(tr_s[j][ns][:, h], tprec,
                                         t8_s[j][ns][:, h])
                else:
                    pprec = tst.tile([P, NF], BF16, name="ppr", tag="ppr")
                    nc.vector.tensor_scalar(pprec, ps, 1.0 / WSCALE,
                                            bp_s[:, fc:fc + 1], ALU.mult,
                                            ALU.add)
                    nc.scalar.activation(p8_s[j][ns][:, h], ps, AF.Identity,
                                         bias=bp_s[:, fc:fc + 1],
                                         scale=1.0 / WSCALE)
                    nc.vector.tensor_sub(pr_s[j][ns][:, h], pprec,
                                         p8_s[j][ns][:, h])

        # phase-1 operands die here; reuse their space for phase-3 x tiles
        ph1cm.__exit__(None, None, None)
        xnp = es.enter_context(tc.tile_pool(name="xnp", bufs=1))
        xn8_s = xnp.tile([P, NCH, D], E4, name="xn8s", tag="xn8s")
        xnr_s = xnp.tile([P, NCH, D], E4, name="xnrs", tag="xnrs")
        nc.sync.dma_start(out=xn8_s, in_=xn8_d[:])
        nc.sync.dma_start(out=xnr_s, in_=xnr_d[:])

        # att8 (and Ar) per m-chunk-pair: phase-3 stationary APs span two
        # adjacent chunks; writes stream per chunk.
        a8_s = [apool.tile([P, 2, N], E4, name=f"a8_{j}", tag=f"a8_{j}")
                for j in range(NCH // 2)]
        if WEIGHTED_TERMS == 3:
            ar_s = [apool.tile([P, 2, N], E4, name=f"ar_{j}", tag=f"ar_{j}")
                    for j in range(NCH // 2)]

        # -------- Phase 2: scores + softmax --------
        # st[m, n] = T.phi + resid terms (= 2*logit - sum_f phi[m, f])
        # E = exp(0.5*st - 20) bf16, one op per ns-PAIR ([128,1024] from a
        # full 2-bank tile), row-sums via accum_out
        # att8 = e4m3(E * recip) on DVE
        for mc in range(NCH):
            sums = stats.tile([P, 2], F32, name="sums", tag="sums")
            e_t = epool.tile([P, N], BF16, name="et", tag="et")
            for nsp in range(NSL // 2):
                pt = ptile()
                for nsh in range(2):
                    ns = 2 * nsp + nsh
                    ps = pt[:, nsh * NF:(nsh + 1) * NF]
                    nmm = 0
                    for j in range(FCP):
                        for (sta, mov) in ((p8_s, t8_s), (pr_s, t8_s),
                                           (p8_s, tr_s)):
                            nc.tensor.matmul(
                                ps, sta[j][mc // 4][:, :, (mc % 4) * P:
                                                    (mc % 4 + 1) * P],
                                mov[j][ns],
                                start=(nmm == 0), stop=(nmm == 5),
                                perf_mode=DR)
                            nmm += 1
                nc.scalar.activation(
                    e_t[:, nsp * 2 * NF:(nsp + 1) * 2 * NF], pt, AF.Exp,
                    bias=eb_s, scale=0.5,
                    accum_out=sums[:, nsp:nsp + 1])
            rs = stats.tile([P, 1], F32, name="rs", tag="rs")
            nc.vector.reduce_sum(rs, sums, axis=AX)
            rc = stats.tile([P, 1], F32, name="rc", tag="rc")
            nc.vector.reciprocal(rc, rs)
            # normalization on DVE: ACT is saturated by the exp stream
            nc.vector.tensor_scalar_mul(a8_s[mc // 2][:, mc % 2], e_t, rc)
            if WEIGHTED_TERMS == 3:
                apre = appool.tile([P, N], BF16, name="ap", tag="ap")
                nc.vector.tensor_scalar_mul(apre, e_t, rc)
                nc.vector.tensor_sub(ar_s[mc // 2][:, mc % 2], apre,
                                     a8_s[mc // 2][:, mc % 2])

        # -------- Phase 3: weighted sum + residual --------
        # out[n, d] = sum_m att[m, n] * x[m, d] + x[n, d]
        nterm = WEIGHTED_TERMS
        for nch in range(NCH):
            nsl128 = slice(nch * P, (nch + 1) * P)
            xrt = xrp.tile([P, D], BF16, name="xrt", tag="xrt")
            nc.sync.dma_start(out=xrt, in_=xr_d[nsl128, :])
            osb = outp.tile([P, D], BF16, name="osb", tag="osb")
            last = (nch == NCH - 1)
            # the very last output runs as 512|384|128 pieces: each piece's
            # add+store overlaps the next piece's matmuls, shrinking the
            # post-PE drain to one narrow add + store
            pieces = [NF, NF - P, P] if last else [NF, NF]
            d0 = 0
            for pi, hw_ in enumerate(pieces):
                dslc = slice(d0, d0 + hw_)
                d0 += hw_
                # every group gets its own (half-used) tile: sharing halves
                # chains each group's start on the other half's consumers
                # through coarsened sem waits
                pt = ptile()
                ps = pt[:, 0:hw_]
                nmm = 0
                for gp in range(NCH // 2):
                    g2 = slice(2 * gp, 2 * gp + 2)
                    pairs = [(a8_s[gp], xn8_s[:, g2, dslc]),
                             (a8_s[gp], xnr_s[:, g2, dslc])]
                    if nterm == 3:
                        pairs.append((ar_s[gp], xn8_s[:, g2, dslc]))
                    for (sta, mov) in pairs:
                        nc.tensor.matmul(ps, sta[:, :, nsl128], mov,
                                         start=(nmm == 0),
                                         stop=(nmm == 8 * nterm - 1),
                                         perf_mode=DR)
                        nmm += 1
                nc.vector.tensor_add(osb[:, dslc], ps, xrt[:, dslc])
                if last:
                    if pi == 0:
                        nc.sync.dma_start(out=out_d[nsl128, dslc],
                                          in_=osb[:, dslc])
                    elif pi == len(pieces) - 1:
                        # one merged store for pieces 1..end: avoids a second
                        # HWDGE slot ahead of the final store's dispatch
                        nc.sync.dma_start(out=out_d[nsl128, NF:],
                                          in_=osb[:, NF:])
            if not last:
                nc.sync.dma_start(out=out_d[nsl128, :], in_=osb)
        es.close()
    nc.finalize()  # Bacc legalization passes (wait splitting, reg alloc, ...)
    return nc


_NC = None


def _get_nc():
    global _NC
    if _NC is None:
        _NC = build_bass()
    return _NC


def _e4(a):
    return np.asarray(a, np.float32).astype(E4NP)


def make_in_maps(x, Wt, bt, Wp, bp):
    def wswz(w, fc):
        # [D, F] -> per-fc [P, DCH, P]: [p, dc, fw] = w[dc*128+p, fc*128+fw]
        blk = w[:, fc * P:(fc + 1) * P]
        return np.ascontiguousarray(blk.reshape(DCH, P, P).transpose(1, 0, 2))

    def wswz_multi(w, fcs):
        # [P, len(fcs), DCH, P] stacked per-fc images
        return np.ascontiguousarray(
            np.stack([wswz(w, fc) for fc in fcs], axis=1))

    def split_w(W):
        wp = WSCALE * np.asarray(W, np.float32)
        w8 = _e4(wp)
        wr = _e4(wp - w8.astype(np.float32))
        return w8, wr

    wt8, wtr8 = split_w(Wt)
    wp8, wpr8 = split_w(Wp)
    fch = bt.size // P
    bt2 = (np.asarray(bt, np.float32) / 2).reshape(fch, P).T
    bp_r = np.asarray(bp, np.float32).reshape(fch, P).T
    bias = np.ascontiguousarray(np.stack([bt2, bp_r], axis=1))

    common = {"bias": bias,
              "wt8r": wswz_multi(wt8, range(1, FCH)),
              "wtr8r": wswz_multi(wtr8, range(1, FCH)),
              "wp8r": wswz_multi(wp8, range(1, FCH)),
              "wpr8r": wswz_multi(wpr8, range(1, FCH))}
    su_w = [wswz(wt8, 0), wswz(wtr8, 0), wswz(wp8, 0), wswz(wpr8, 0)]

    def xtimg(a):  # [N, D] e4m3 -> per-ns [P, DCH, NF] images of a.T
        at = np.ascontiguousarray(a.T)         # [D, N]
        r = at.reshape(DCH, P, N)
        return [np.ascontiguousarray(r[:, :, ns * NF:(ns + 1) * NF]
                                     .transpose(1, 0, 2))
                for ns in range(NSL)]

    def xnimg(a):  # [N, D] e4m3 -> [P, NCH, D]
        return np.ascontiguousarray(
            a.reshape(NCH, P, D).transpose(1, 0, 2))

    in_maps = []
    for b in range(x.shape[0]):
        xb = np.ascontiguousarray(np.asarray(x[b], np.float32))
        x8 = _e4(xb)
        xr8 = _e4(xb - x8.astype(np.float32))
        m = dict(common)
        xt_imgs = xtimg(x8)
        xtr_imgs = xtimg(xr8)
        # startup image: [wt80|wtr80|wp80|wpr80|xt80] per dc row
        m["su"] = np.ascontiguousarray(
            np.concatenate(su_w + [xt_imgs[0]], axis=2))
        m["xtr80_"] = xtr_imgs[0]
        for ns in range(1, NSL):
            m[f"xt8{ns}"] = xt_imgs[ns]
            m[f"xtr8{ns}"] = xtr_imgs[ns]
        m["xn8"] = xnimg(x8)
        m["xnr8"] = xnimg(xr8)
        m["xr"] = xb.astype(ml_dtypes.bfloat16)
        in_maps.append(m)
    return in_maps


def run(inputs, trace=False):
    """Run on 8 NeuronCores; returns (out [B,N,D] f32, BassKernelResults)."""
    x = inputs["x"]
    assert x.shape == (B, N, D), x.shape
    nc = _get_nc()
    in_maps = make_in_maps(x, inputs["Wt"], inputs["bt"], inputs["Wp"],
                           inputs["bp"])
    res = run_bass_kernel_spmd(nc, in_maps, core_ids=list(range(B)),
                               trace=trace)
    out = np.stack([res.results[c]["out"] for c in range(B)], axis=0)
    return out.astype(np.float32), res


def kernel(**inputs) -> np.ndarray:
    out, _ = run(inputs)
    return out


# revision 64
# speedup vs baseline: 1.0045x; 1.0045x over previous
"""Trainium2 Bass kernel for nn_AttentionModule (dense_transformer), fp8 DR.

Reference computation (per batch sample b):
    theta = sigmoid(x @ Wt + bt)            # [N, F]
    phi   = x @ Wp + bp                     # [N, F]
    att   = theta @ phi.T                   # [N(n), N(m)]
    att   = softmax(att, axis over n)       # softmax over QUERY axis
    out   = att(n,m) @ x(m,d) + x           # [N, D]

Strategy: pure data parallelism (B=8 samples, one per core, no
collectives) + fp8-e4m3 DoubleRow matmuls (0.5 cycles/row, 2x128
contraction per instruction = 4x bf16 MAC rate in the cost model).

Precision scheme (validated in numerics2.py, rel-l2 vs f64 ref):
 - every bf16 matmul X@W is replaced by 2-3 fp8 terms
       X8@W8 + X8@Wr + Xr@W8      (r = unscaled e4m3 residual)
   accumulated in the same fp32 PSUM group; residual pairs restore
   ~bf16-equivalent precision at 0.5-0.75x of bf16 matmul time.
 - weights are pre-scaled by 32 on host (W' = 32W) so their e4m3
   residuals land in normal range; the 1/32 is folded into the
   activation scale.
 - theta is carried as T = tanh(z/2) = 2*sigmoid(z)-1 in (-1,1):
   the scores logit becomes 0.5*(T.phi) + 0.5*sum_f(phi[m]); the
   second term is constant per m and cancels in the softmax over n
   (exp bias -20; measured exponent range on real inputs [-8, 11.1]).
 - att is quantized to e4m3 AFTER normalization; weighted uses
   2 terms (att8@x8 + att8@xr8, rel~1.38e-2) or 3 terms (+Ar@x8,
   rel~5.5e-3) per WEIGHTED_TERMS.

Scheduling (engine budgets from the TimelineSim cost model):
 - PE (full clock): proj 41us + scores 41us + weighted 55us (2t).
 - PSUM tiles are 2 banks wide; two 512-wide accumulation groups
   share a tile so phase-2 exp runs once per ns-PAIR ([128,1024]
   reads). This halves the per-exp fixed costs (172-cycle PSUM
   access + 187ns accumulator read) that otherwise pace phase 2
   above PE rate (ACT was 799ns/group vs PE 642ns).
 - exp uses accum_out for free row-sums; normalization (att8) runs
   on DVE; residual prep is split ACT/DVE so both stay under PE.
 - tiles are split to consumer granularity (deps are tile-granular):
   W per fc (startup pipelining), T/P per (fc-pair, ns), att8/Ar per
   m-chunk-pair, E per chunk (rotating pool).
 - SBUF: the phase-1 operand pool is closed after phase 1 and its
   space reused for the phase-3 x tiles (LIFO pool stack).
"""

import numpy as np
import ml_dtypes

import concourse.bass as bass
import concourse.bacc as bacc
import concourse.mybir as mybir
from concourse.tile import TileContext
from concourse.bass_utils import run_bass_kernel_spmd

P = 128
B, N, D, F = 8, 2048, 1024, 512
NCH = N // P    # 16 m/n chunks
DCH = D // P    # 8 d chunks
FCH = F // P    # 4 f chunks
NF = 512        # accumulation-group width (half a 2-bank psum tile)
NSL = N // NF   # 4 column slices
DSL = D // NF   # 2 output d slices
WSCALE = 32.0   # host weight pre-scale (residuals out of denormals)

WEIGHTED_TERMS = 2   # 2: att8@(x8+xr8) ~1.4e-2 | 3: +Ar@x8 ~5.5e-3

BF16 = mybir.dt.bfloat16
F32 = mybir.dt.float32
E4 = mybir.dt.float8e4
AX = mybir.AxisListType.X
AF = mybir.ActivationFunctionType
DR = mybir.MatmulPerfMode.DoubleRow
ALU = mybir.AluOpType
E4NP = ml_dtypes.float8_e4m3


def build_bass():
    nc = bacc.Bacc()

    # startup image: what the first (fc0, ns0) groups' main+wres terms need,
    # in ONE DMA: [wt80|wtr80|wp80|wpr80|xt80] packed per dc row. xtr80
    # (the xres operand, LAST term of each group) rides separately: the
    # scheduler's wait-queue bypass covers ~0.9us of main/wres work while
    # it lands, so phase 1 starts ~1.2us earlier.
    SUW = 4 * P + NF
    su_d = nc.declare_dram_parameter("su", [P, DCH, SUW], E4, isOutput=False)
    xtr0_d = nc.declare_dram_parameter("xtr80_", [P, DCH, NF], E4,
                                       isOutput=False)
    wt8r_d = nc.declare_dram_parameter("wt8r", [P, FCH - 1, DCH, P], E4,
                                       isOutput=False)
    wtrr_d = nc.declare_dram_parameter("wtr8r", [P, FCH - 1, DCH, P], E4,
                                       isOutput=False)
    wp8r_d = nc.declare_dram_parameter("wp8r", [P, FCH - 1, DCH, P], E4,
                                       isOutput=False)
    wprr_d = nc.declare_dram_parameter("wpr8r", [P, FCH - 1, DCH, P], E4,
                                       isOutput=False)
    bias_d = nc.declare_dram_parameter("bias", [P, 2, FCH], F32,
                                       isOutput=False)
    xt8_d = [nc.declare_dram_parameter(f"xt8{ns}", [P, DCH, NF], E4,
                                       isOutput=False) for ns in range(1, NSL)]
    xtr_d = [nc.declare_dram_parameter(f"xtr8{ns}", [P, DCH, NF], E4,
                                       isOutput=False) for ns in range(1, NSL)]
    xn8_d = nc.declare_dram_parameter("xn8", [P, NCH, D], E4, isOutput=False)
    xnr_d = nc.declare_dram_parameter("xnr8", [P, NCH, D], E4, isOutput=False)
    xr_d = nc.declare_dram_parameter("xr", [N, D], BF16, isOutput=False)
    # bf16 output (upcast on host): halves the output DMA traffic and the
    # post-PE drain; adds ~2e-4 rel error (negligible vs the 1.4e-2 total)
    out_d = nc.declare_dram_parameter("out", [N, D], BF16, isOutput=True)

    with TileContext(nc) as tc:
        from contextlib import ExitStack
        es = ExitStack()
        cpool = es.enter_context(tc.tile_pool(name="const", bufs=1))
        stats = es.enter_context(tc.tile_pool(name="stats", bufs=8))
        epool = es.enter_context(tc.tile_pool(name="ep", bufs=8))
        apool = es.enter_context(tc.tile_pool(name="a8", bufs=1))
        appool = es.enter_context(tc.tile_pool(name="apre", bufs=2))
        xrp = es.enter_context(tc.tile_pool(name="xrp", bufs=3))
        outp = es.enter_context(tc.tile_pool(name="outp", bufs=3))
        tpp = es.enter_context(tc.tile_pool(name="tpp", bufs=1))
        tst = es.enter_context(tc.tile_pool(name="tst", bufs=6))
        # 2-bank psum tiles, two 512-wide groups per tile (zero regions are
        # 2KB, so each half is an independent accumulation group)
        psum = es.enter_context(tc.tile_pool(name="psum", bufs=4,
                                             space="PSUM"))
        ph1cm = tc.tile_pool(name="ph1", bufs=1)
        ph1 = ph1cm.__enter__()

        def ptile():
            return psum.tile([P, 2 * NF], F32, name="pst", tag="pst")

        # ---- constants ----
        bias_s = cpool.tile([P, 2, FCH], F32, name="bias", tag="bias")
        bt2_s = bias_s[:, 0]
        bp_s = bias_s[:, 1]
        zx = cpool.tile([P, P], BF16, name="zx", tag="zx")
        nc.vector.memset(zx, 0)

        # PE warm-up: the pstate ramp holds PE below 2.4GHz for ~3.4us of
        # sustained activity; the first real matmul waits on DMA anyway, so
        # burn the idle time on dummy matmuls (costless: PE was idle).
        NWARM = 46  # 128-wide dummies: cover the startup DMA latency; slight
        # overshoot is safer than a gap (a PE gap resets the pstate ramp)
        zp = ptile()
        for i in range(NWARM):
            nc.tensor.matmul(zp[:, 0:P], zx, zx, start=(i == 0),
                             stop=(i == NWARM - 1))
        eb_s = cpool.tile([P, 1], F32, name="ebs", tag="ebs")
        nc.vector.memset(eb_s, -20.0)

        # ---- phase-1 operand tiles + DMAs (first-use order) ----
        su_s = ph1.tile([P, DCH, SUW], E4, name="sus", tag="sus")
        xtr0_s = ph1.tile([P, DCH, NF], E4, name="xtr0s", tag="xtr0s")
        wt8r_s = ph1.tile([P, FCH - 1, DCH, P], E4, name="wt8rs", tag="wt8rs")
        wtrr_s = ph1.tile([P, FCH - 1, DCH, P], E4, name="wtrrs", tag="wtrrs")
        wp8r_s = ph1.tile([P, FCH - 1, DCH, P], E4, name="wp8rs", tag="wp8rs")
        wprr_s = ph1.tile([P, FCH - 1, DCH, P], E4, name="wprrs", tag="wprrs")
        xt8_s = [None] + [ph1.tile([P, DCH, NF], E4, name=f"xt8{ns}",
                                   tag=f"xt8{ns}") for ns in range(1, NSL)]
        xtr_s = [None] + [ph1.tile([P, DCH, NF], E4, name=f"xtr{ns}",
                                   tag=f"xtr{ns}") for ns in range(1, NSL)]

        def wt_ap(fc, s2):
            return su_s[:, s2, 0:P] if fc == 0 else wt8r_s[:, fc - 1, s2]

        def wtr_ap(fc, s2):
            return (su_s[:, s2, P:2 * P] if fc == 0
                    else wtrr_s[:, fc - 1, s2])

        def wp_ap(fc, s2):
            return (su_s[:, s2, 2 * P:3 * P] if fc == 0
                    else wp8r_s[:, fc - 1, s2])

        def wpr_ap(fc, s2):
            return (su_s[:, s2, 3 * P:4 * P] if fc == 0
                    else wprr_s[:, fc - 1, s2])

        def xt_mov(ns, s2):
            return (su_s[:, s2, 4 * P:4 * P + NF] if ns == 0
                    else xt8_s[ns][:, s2])

        def xtr_mov(ns, s2):
            return xtr0_s[:, s2] if ns == 0 else xtr_s[ns][:, s2]

        nc.sync.dma_start(out=su_s, in_=su_d[:])
        nc.sync.dma_start(out=xtr0_s, in_=xtr0_d[:])
        nc.sync.dma_start(out=bias_s, in_=bias_d[:])
        nc.sync.dma_start(out=wt8r_s, in_=wt8r_d[:])
        nc.sync.dma_start(out=wtrr_s, in_=wtrr_d[:])
        nc.sync.dma_start(out=wp8r_s, in_=wp8r_d[:])
        nc.sync.dma_start(out=wprr_s, in_=wprr_d[:])
        for ns in range(1, NSL):
            nc.sync.dma_start(out=xt8_s[ns], in_=xt8_d[ns - 1][:])
            nc.sync.dma_start(out=xtr_s[ns], in_=xtr_d[ns - 1][:])

        # T/P operand tiles: per (fc-pair, ns-512) so scores(mc, ns) only
        # waits on the exact phase-1 blocks it reads.
        FCP = FCH // 2
        t8_s = [[tpp.tile([P, 2, NF], E4, name=f"t8_{j}_{ns}",
                          tag=f"t8_{j}_{ns}") for ns in range(NSL)]
                for j in range(FCP)]
        tr_s = [[tpp.tile([P, 2, NF], E4, name=f"tr_{j}_{ns}",
                          tag=f"tr_{j}_{ns}") for ns in range(NSL)]
                for j in range(FCP)]
        p8_s = [[tpp.tile([P, 2, NF], E4, name=f"p8_{j}_{ns}",
                          tag=f"p8_{j}_{ns}") for ns in range(NSL)]
                for j in range(FCP)]
        pr_s = [[tpp.tile([P, 2, NF], E4, name=f"pr_{j}_{ns}",
                          tag=f"pr_{j}_{ns}") for ns in range(NSL)]
                for j in range(FCP)]

        # -------- Phase 1: projections (3-term fp8 DR) --------
        # psum = x8@W8' + x8@Wr' + xr8@W8'   (W' = 32W; 12 DR per group)
        # T = tanh(psum/64 + bt/2)  -> bf16 staging + e4m3 + residual
        # phi = psum/32 + bp        -> same
        pt = None
        NS0_ORDER = [("t", 0), ("p", 0), ("t", 1), ("t", 2), ("p", 1),
                     ("t", 3), ("p", 2), ("p", 3)]
        for ns in range(NSL):
            # theta/phi interleaved: ACT sees tanh,tanh,id,... instead of an
            # 8-tanh bunch followed by 4 ids — smoother consumer flow for the
            # psum-tile rotation. ns0 is ordered by DMA arrival instead: the
            # theta weights land before the phi ones, so theta groups fill
            # the window while wp8r/wpr8r stream in.
            for gi, (proj, fc) in enumerate(
                    NS0_ORDER if ns == 0 else
                    [(pr_, fc) for fc in range(FCH) for pr_ in ("t", "p")]):
                pt = ptile()
                ps = pt[:, 0:NF]
                wa, wra = ((wt_ap, wtr_ap) if proj == "t"
                           else (wp_ap, wpr_ap))
                # term-major order: all main-term matmuls first, so the
                # startup groups run while the residual operands' DMAs land
                nmm = 0
                for (wf, xf) in ((wa, xt_mov), (wra, xt_mov),
                                 (wa, xtr_mov)):
                    for dcp in range(DCH // 2):
                        s2 = slice(2 * dcp, 2 * dcp + 2)
                        nc.tensor.matmul(ps, wf(fc, s2), xf(ns, s2),
                                         start=(nmm == 0), stop=(nmm == 11),
                                         perf_mode=DR)
                        nmm += 1
                j, h = fc // 2, fc % 2
                if proj == "t":
                    tprec = tst.tile([P, NF], BF16, name="tpr", tag="tpr")
                    nc.scalar.activation(tprec, ps, AF.Tanh,
                                         bias=bt2_s[:, fc:fc + 1],
                                         scale=1.0 / (2 * WSCALE))
                    # t8 from the bf16 staging on DVE (not a 2nd ACT tanh):
                    # frees the psum bank after ONE ACT pass
                    nc.vector.tensor_copy(t8_s[j][ns][:, h], tprec)
                    nc.vector.tensor_sub(tr_s[j][ns][:, h], tprec,
                                         t8_s[j][ns][:, h])
                else:
                    pprec = tst.tile([P, NF], BF16, name="ppr", tag="ppr")
                    nc.vector.tensor_scalar(pprec, ps, 1.0 / WSCALE,
                                            bp_s[:, fc:fc + 1], ALU.mult,
                                            ALU.add)
                    nc.scalar.activation(p8_s[j][ns][:, h], ps, AF.Identity,
                                         bias=bp_s[:, fc:fc + 1],
                                         scale=1.0 / WSCALE)
                    nc.vector.tensor_sub(pr_s[j][ns][:, h], pprec,
                                         p8_s[j][ns][:, h])

        # phase-1 operands die here; reuse their space for phase-3 x tiles
        ph1cm.__exit__(None, None, None)
        xnp = es.enter_context(tc.tile_pool(name="xnp", bufs=1))
        xn8_s = xnp.tile([P, NCH, D], E4, name="xn8s", tag="xn8s")
        xnr_s = xnp.tile([P, NCH, D], E4, name="xnrs", tag="xnrs")
        nc.sync.dma_start(out=xn8_s, in_=xn8_d[:])
        nc.sync.dma_start(out=xnr_s, in_=xnr_d[:])

        # att8 (and Ar) per m-chunk-pair: phase-3 stationary APs span two
        # adjacent chunks; writes stream per chunk.
        a8_s = [apool.tile([P, 2, N], E4, name=f"a8_{j}", tag=f"a8_{j}")
                for j in range(NCH // 2)]
        if WEIGHTED_TERMS == 3:
            ar_s = [apool.tile([P, 2, N], E4, name=f"ar_{j}", tag=f"ar_{j}")
                    for j in range(NCH // 2)]

        # -------- Phase 2: scores + softmax --------
        # st[m, n] = T.phi + resid terms (= 2*logit - sum_f phi[m, f])
        # E = exp(0.5*st - 20) bf16, one op per ns-PAIR ([128,1024] from a
        # full 2-bank tile), row-sums via accum_out
        # att8 = e4m3(E * recip) on DVE
        for mc in range(NCH):
            sums = stats.tile([P, 2], F32, name="sums", tag="sums")
            e_t = epool.tile([P, N], BF16, name="et", tag="et")
            for nsp in range(NSL // 2):
                pt = ptile()
                for nsh in range(2):
                    ns = 2 * nsp + nsh
                    ps = pt[:, nsh * NF:(nsh + 1) * NF]
                    nmm = 0
                    for j in range(FCP):
                        for (sta, mov) in ((p8_s, t8_s), (pr_s, t8_s),
                                           (p8_s, tr_s)):
                            nc.tensor.matmul(
                                ps, sta[j][mc // 4][:, :, (mc % 4) * P:
                                                    (mc % 4 + 1) * P],
                                mov[j][ns],
                                start=(nmm == 0), stop=(nmm == 5),
                                perf_mode=DR)
                            nmm += 1
                nc.scalar.activation(
                    e_t[:, nsp * 2 * NF:(nsp + 1) * 2 * NF], pt, AF.Exp,
                    bias=eb_s, scale=0.5,
                    accum_out=sums[:, nsp:nsp + 1])
            rs = stats.tile([P, 1], F32, name="rs", tag="rs")
            nc.vector.reduce_sum(rs, sums, axis=AX)
            rc = stats.tile([P, 1], F32, name="rc", tag="rc")
            nc.vector.reciprocal(rc, rs)
            # normalization on DVE: ACT is saturated by the exp stream
            nc.vector.tensor_scalar_mul(a8_s[mc // 2][:, mc % 2], e_t, rc)
            if WEIGHTED_TERMS == 3:
                apre = appool.tile([P, N], BF16, name="ap", tag="ap")
                nc.vector.tensor_scalar_mul(apre, e_t, rc)
                nc.vector.tensor_sub(ar_s[mc // 2][:, mc % 2], apre,
                                     a8_s[mc // 2][:, mc % 2])

        # -------- Phase 3: weighted sum + residual --------
        # out[n, d] = sum_m att[m, n] * x[m, d] + x[n, d]
        nterm = WEIGHTED_TERMS
        for nch in range(NCH):
            nsl128 = slice(nch * P, (nch + 1) * P)
            xrt = xrp.tile([P, D], BF16, name="xrt", tag="xrt")
            nc.sync.dma_start(out=xrt, in_=xr_d[nsl128, :])
            osb = outp.tile([P, D], BF16, name="osb", tag="osb")
            last = (nch == NCH - 1)
            # the very last output runs as 512|384|128 pieces: each piece's
            # add+store overlaps the next piece's matmuls, shrinking the
            # post-PE drain to one narrow add + store
            pieces = [NF, NF - P, P] if last else [NF, NF]
            d0 = 0
            for pi, hw_ in enumerate(pieces):
                dslc = slice(d0, d0 + hw_)
                d0 += hw_
                # every group gets its own (half-used) tile: sharing halves
                # chains each group's start on the other half's consumers
                # through coarsened sem waits
                pt = ptile()
                ps = pt[:, 0:hw_]
                nmm = 0
                for gp in range(NCH // 2):
                    g2 = slice(2 * gp, 2 * gp + 2)
                    pairs = [(a8_s[gp], xn8_s[:, g2, dslc]),
                             (a8_s[gp], xnr_s[:, g2, dslc])]
                    if nterm == 3:
                        pairs.append((ar_s[gp], xn8_s[:, g2, dslc]))
                    for (sta, mov) in pairs:
                        nc.tensor.matmul(ps, sta[:, :, nsl128], mov,
                                         start=(nmm == 0),
                                         stop=(nmm == 8 * nterm - 1),
                                         perf_mode=DR)
                        nmm += 1
                nc.vector.tensor_add(osb[:, dslc], ps, xrt[:, dslc])
                if last:
                    if pi == 0:
                        nc.sync.dma_start(out=out_d[nsl128, dslc],
                                          in_=osb[:, dslc])
                    elif pi == len(pieces) - 1:
                        # one merged store for pieces 1..end: avoids a second
                        # HWDGE slot ahead of the final store's dispatch
                        nc.sync.dma_start(out=out_d[nsl128, NF:],
                                          in_=osb[:, NF:])
            if not last:
                nc.sync.dma_start(out=out_d[nsl128, :], in_=osb)
        es.close()
    nc.finalize()  # Bacc legalization passes (wait splitting, reg alloc, ...)
    return nc


_NC = None


def _get_nc():
    global _NC
    if _NC is None:
        _NC = build_bass()
    return _NC


def _e4(a):
    return np.asarray(a, np.float32).astype(E4NP)


def make_in_maps(x, Wt, bt, Wp, bp):
    def wswz(w, fc):
        # [D, F] -> per-fc [P, DCH, P]: [p, dc, fw] = w[dc*128+p, fc*128+fw]
        blk = w[:, fc * P:(fc + 1) * P]
        return np.ascontiguousarray(blk.reshape(DCH, P, P).transpose(1, 0, 2))

    def wswz_multi(w, fcs):
        # [P, len(fcs), DCH, P] stacked per-fc images
        return np.ascontiguousarray(
            np.stack([wswz(w, fc) for fc in fcs], axis=1))

    def split_w(W):
        wp = WSCALE * np.asarray(W, np.float32)
        w8 = _e4(wp)
        wr = _e4(wp - w8.astype(np.float32))
        return w8, wr

    wt8, wtr8 = split_w(Wt)
    wp8, wpr8 = split_w(Wp)
    fch = bt.size // P
    bt2 = (np.asarray(bt, np.float32) / 2).reshape(fch, P).T
    bp_r = np.asarray(bp, np.float32).reshape(fch, P).T
    bias = np.ascontiguousarray(np.stack([bt2, bp_r], axis=1))

    common = {"bias": bias,
              "wt8r": wswz_multi(wt8, range(1, FCH)),
              "wtr8r": wswz_multi(wtr8, range(1, FCH)),
              "wp8r": wswz_multi(wp8, range(1, FCH)),
              "wpr8r": wswz_multi(wpr8, range(1, FCH))}
    su_w = [wswz(wt8, 0), wswz(wtr8, 0), wswz(wp8, 0), wswz(wpr8, 0)]

    def xtimg(a):  # [N, D] e4m3 -> per-ns [P, DCH, NF] images of a.T
        at = np.ascontiguousarray(a.T)         # [D, N]
        r = at.reshape(DCH, P, N)
        return [np.ascontiguousarray(r[:, :, ns * NF:(ns + 1) * NF]
                                     .transpose(1, 0, 2))
                for ns in range(NSL)]

    def xnimg(a):  # [N, D] e4m3 -> [P, NCH, D]
        return np.ascontiguousarray(
            a.reshape(NCH, P, D).transpose(1, 0, 2))

    in_maps = []
    for b in range(x.shape[0]):
        xb = np.ascontiguousarray(np.asarray(x[b], np.float32))
        x8 = _e4(xb)
        xr8 = _e4(xb - x8.astype(np.float32))
        m = dict(common)
        xt_imgs = xtimg(x8)
        xtr_imgs = xtimg(xr8)
        # startup image: [wt80|wtr80|wp80|wpr80|xt80] per dc row
        m["su"] = np.ascontiguousarray(
            np.concatenate(su_w + [xt_imgs[0]], axis=2))
        m["xtr80_"] = xtr_imgs[0]
        for ns in range(1, NSL):
            m[f"xt8{ns}"] = xt_imgs[ns]
            m[f"xtr8{ns}"] = xtr_imgs[ns]
        m["xn8"] = xnimg(x8)
        m["xnr8"] = xnimg(xr8)
        m["xr"] = xb.astype(ml_dtypes.bfloat16)
        in_maps.append(m)
    return in_maps


def run(inputs, trace=False):
    """Run on 8 NeuronCores; returns (out [B,N,D] f32, BassKernelResults)."""
    x = inputs["x"]
    assert x.shape == (B, N, D), x.shape
    nc = _get_nc()
    in_maps = make_in_maps(x, inputs["Wt"], inputs["bt"], inputs["Wp"],
                           inputs["bp"])
    res = run_bass_kernel_spmd(nc, in_maps, core_ids=list(range(B)),
                               trace=trace)
    out = np.stack([res.results[c]["out"] for c in range(B)], axis=0)
    return out.astype(np.float32), res


def kernel(**inputs) -> np.ndarray:
    out, _ = run(inputs)
    return out
